# revision 2
# baseline (speedup 1.0000x reference)
# Trainium2 Bass kernel for GQA attention block (B=8, T=512, C=2048, 16 q heads,
# 4 kv heads, head_dim=128, RoPE, causal mask, output projection).
#
# Strategy: data parallel across the 8 NeuronCores — core i handles batch
# element i with the full weight set (no collectives). Per core everything is
# computed in a transposed layout:
#   qT/kT [d, t]  = W[:, d-tile].T-contract  (lhsT = weight tile, rhs = xT)
#   scoresT [s,t] = kT-slice.T @ qT          (softmax normalizer along the
#                                             partition dim via a ones-matmul
#                                             that broadcasts the sum to all
#                                             128 partitions for free)
#   outT [d, t]   = v-slice.T @ (mask*exp(scoresT))  (unnormalized)
#   y [t, e]      = outT-slice.T @ wo-tile   (normalized outT)
# RoPE rotate-half is a signed 128x128 permutation matmul + 2 muls + 1 add.
# Matmul operands are fp16 (full PE rate, half the HBM bytes of fp32, DVE 2x
# modes; fp32 PSUM accumulation throughout).
#
# Phase A streams v-proj and k-proj interleaved per 128-row contraction tile:
# each ct consumes x[ct] (scalar ring) + wv[ct] (sync ring) + wk[ct] (gpsimd
# ring) — ~226 GB/s aggregate, under the 358 GB/s per-core HBM cap, with each
# 128KB chunk arriving just-in-time (k lags v by 2 cts to cover the slower
# SWDGE first-chunk latency). kv heads 0/1 finish their contraction two cts
# before heads 2/3, so rope-k0/k1 (DVE) runs under the k23 tail matmuls and
# the first q-head projection starts with no PE bubble. When the runtime mask
# is exactly causal, the scores/AV/normalizer matmuls restrict their moving
# dim to the visible t-range and only the diagonal 128x128 block gets
# mask-multiplied; any other mask falls back to a general masked build.

import os
import sys

import numpy as np

for _p in (
    "/root/.axon_site",
    "/root/.axon_site/_ro/trn_rl_repo",
    "/root/.axon_site/_ro/pypackages",
    "/opt/trn_rl_repo",
):
    if os.path.isdir(_p) and _p not in sys.path:
        sys.path.append(_p)

import concourse.bass as bass  # noqa: E402
import concourse.mybir as mybir  # noqa: E402
import concourse.tile as tile  # noqa: E402
from concourse import bacc  # noqa: E402
from concourse.bass_utils import run_bass_kernel_spmd  # noqa: E402

F32 = mybir.dt.float32
F32R = mybir.dt.float32r
F16 = mybir.dt.float16
AF = mybir.ActivationFunctionType

B, T, C = 8, 512, 2048
HD, NH, NKV = 128, 16, 4
CT = C // 128  # 16 contraction tiles
TT = T // 128  # 4 t/s tiles
EG = C // 512  # 4 output column groups
REP = NH // NKV
SCALE = float(HD) ** -0.5
N_CORES = 8

_DT_ENV = os.environ.get("ATTN_DTYPE", "fp16")
MM_DT = {"fp16": F16, "fp32r": F32R, "fp32": F32}[_DT_ENV]
MM_NP = {"fp16": np.float16, "fp32r": np.float32, "fp32": np.float32}[_DT_ENV]


def _s(i, n):
    return slice(i * n, (i + 1) * n)


def _emit(tc, xT, wq, wk, wv, wo, cosT, sinT, maskT, y, causal):
    nc = tc.nc
    mm = nc.tensor.matmul

    with (
        tc.tile_pool(name="consts", bufs=1) as consts,
        tc.tile_pool(name="streams", bufs=2) as streams,
        tc.tile_pool(name="work", bufs=2) as work,
        tc.tile_pool(name="ps", bufs=1, space="PSUM") as ps,
    ):
        cosT_sb = consts.tile([HD, T], MM_DT)
        sinT_sb = consts.tile([HD, T], MM_DT)
        # causal: one [128,128] upper-tri block reused for every diagonal
        # tile; general: the full [s,t] mask
        maskT_sb = consts.tile([128, 128] if causal else [128, TT, T], MM_DT)
        ones_sb = consts.tile([128, 128], MM_DT)
        xT_sb = consts.tile([128, CT, T], MM_DT)
        kT_sb = consts.tile([HD, NKV, T], MM_DT)
        v_sb = consts.tile([128, TT, 4 * HD], MM_DT)
        aout_sb = consts.tile([HD, NH, T], MM_DT)

        # ---- PE warmup: the HAM clock gate keeps the PE at half rate until
        # a ~3.4us activity window of sustained matmuls. A short burst covers
        # the DMA-latency head (first real operands land ~1.5us after issue);
        # from there the gapless phase-A stream keeps the window busy.
        warm_sb = consts.tile([128, T], MM_DT)
        nc.gpsimd.memset(warm_sb[:], 0.0)
        nc.gpsimd.memset(ones_sb[:], 1.0)
        wps = ps.tile([HD, T], F32, tag="qp", bufs=2, name="warmps")
        for _ in range(4):
            mm(wps[:], warm_sb[:, :128], warm_sb[:], start=True, stop=True)

        # ---- DMA schedule: one stream per ring, chunks sized so each lands
        # just before its consumption point (a DMA's semaphore only fires
        # when the WHOLE chunk lands; early chunks are single-ct).
        # scalar ring: x, then the first wq heads.
        for c0, ncs in ((0, 1), (1, 1), (2, 1), (3, 1), (4, 2), (6, 2), (8, 4), (12, 4)):
            nc.scalar.dma_start(
                xT_sb[:, c0 : c0 + ncs, :],
                xT[128 * c0 : 128 * (c0 + ncs), :].rearrange("(c p) t -> p c t", p=128),
            )

        # sync ring: wv per-ct chunks.
        wvt = {}
        for c0, ncs in ((0, 1), (1, 1), (2, 1), (3, 1), (4, 2), (6, 2), (8, 2), (10, 2), (12, 2), (14, 2)):
            t = streams.tile([128, 2, 4 * HD], MM_DT, tag="wv", bufs=10, name=f"wv{c0}")
            nc.sync.dma_start(
                t[:, :ncs, :],
                wv[c0 * 128 : (c0 + ncs) * 128, :].rearrange("(c p) d -> p c d", p=128),
            )
            for ci in range(ncs):
                wvt[c0 + ci] = (t, ci)

        # gpsimd ring: wk chunks, then the small consts (needed ~25us in).
        wkt = {}
        for c0, ncs in ((0, 1), (1, 1), (2, 2), (4, 2), (6, 2), (8, 2), (10, 2), (12, 2), (14, 2)):
            t = streams.tile([128, 2, 4 * HD], MM_DT, tag="wk", bufs=9, name=f"wk{c0}")
            nc.gpsimd.dma_start(
                t[:, :ncs, :],
                wk[c0 * 128 : (c0 + ncs) * 128, :].rearrange("(c p) d -> p c d", p=128),
            )
            for ci in range(ncs):
                wkt[c0 + ci] = (t, ci)
        nc.gpsimd.dma_start(cosT_sb[:], cosT)
        nc.gpsimd.dma_start(sinT_sb[:], sinT)
        if causal:
            nc.gpsimd.dma_start(maskT_sb[:], maskT)
        else:
            nc.gpsimd.dma_start(
                maskT_sb[:], maskT.rearrange("(st p) t -> p st t", p=128)
            )

        # wq head-slice prefetch, alternating between the two HWDGE rings
        wqh_tiles = {}

        def wqh_dma(h, eng=None):
            if h >= NH:
                return
            wqh = streams.tile([128, CT, HD], MM_DT, tag="wqh", bufs=6, name=f"wqh{h}")
            (eng or (nc.scalar if h % 2 == 0 else nc.sync)).dma_start(
                wqh[:], wq[:, _s(h, HD)].rearrange("(ct p) d -> p ct d", p=128)
            )
            wqh_tiles[h] = wqh

        # ---- rope helper: psrc (PSUM, [d, t] f32) -> out_slice (SBUF) ----
        # rotate-half via partition-shifted PSUM reads; sinT_sb rows 0:64 are
        # pre-negated on the host, so no rotation matmul is needed.
        def rope(psrc, out_slice, tag):
            qrot = work.tile([HD, T], MM_DT, tag="trot", name=f"qrot_{tag}")
            nc.scalar.copy(qrot[0:64, :], psrc[64:128, :])
            nc.scalar.copy(qrot[64:128, :], psrc[0:64, :])
            tcos = work.tile([HD, T], MM_DT, tag="tcos", name=f"tcos_{tag}")
            nc.vector.tensor_mul(tcos[:], psrc, cosT_sb[:])
            nc.vector.tensor_mul(qrot[:], qrot[:], sinT_sb[:])
            nc.vector.tensor_add(out_slice, tcos[:], qrot[:])

        # PSUM "half" tiles: 2 banks each, double-buffered
        def half_pair(name):
            a = ps.tile([128, 2, 512], F32, tag="sthalf", bufs=2, name=f"{name}a")
            b = ps.tile([128, 2, 512], F32, tag="sthalf", bufs=2, name=f"{name}b")
            return a, b

        # ---- phase A: v and k projections interleaved per ct ----
        vp_a, vp_b = half_pair("vp")
        vps = [vp_a[:, 0, :], vp_a[:, 1, :], vp_b[:, 0, :], vp_b[:, 1, :]]
        _kp_tags = (("qp", 2), ("qp", 2), ("av", 1), ("lsum", 1))
        kps = [
            ps.tile([HD, T], F32, tag=t, bufs=bf, name=f"kp{j}")
            for j, (t, bf) in enumerate(_kp_tags)
        ]

        def vmm(ct):
            t, ci = wvt[ct]
            for i in range(TT):
                mm(
                    vps[i],
                    xT_sb[:, ct, _s(i, 128)],
                    t[:, ci, :],
                    start=(ct == 0),
                    stop=(ct == CT - 1),
                )

        def kmm(ct, js):
            t, ci = wkt[ct]
            for j in js:
                mm(
                    kps[j][:],
                    t[:, ci, _s(j, HD)],
                    xT_sb[:, ct, :],
                    start=(ct == 0),
                    stop=(ct == CT - 1),
                )

        for ct in range(CT):
            vmm(ct)
            if ct >= 2:
                kmm(ct - 2, (0, 1))
            if ct >= 4:
                kmm(ct - 4, (2, 3))
        kmm(14, (0, 1))
        kmm(15, (0, 1))
        # rope k0/k1 (DVE/ACT) runs under the k23 tail matmuls, freeing the
        # qp psum banks before the first q-head projection needs them
        rope(kps[0][:], kT_sb[:, 0, :], "k0")
        rope(kps[1][:], kT_sb[:, 1, :], "k1")
        for ct in (12, 13, 14, 15):
            kmm(ct, (2, 3))
        for i in range(TT):
            nc.vector.tensor_copy(v_sb[:, i, :], vps[i])
        for h in range(3):
            wqh_dma(h)

        # ---- phase B: per q head, software pipelined ----
        state = {}

        def stage_a(h):  # projection matmuls into psum
            qp = ps.tile([HD, T], F32, tag="qp", bufs=2, name=f"qp{h}")
            wqh = wqh_tiles.pop(h)
            for ct in range(CT):
                mm(
                    qp[:],
                    wqh[:, ct, :],
                    xT_sb[:, ct, :],
                    start=(ct == 0),
                    stop=(ct == CT - 1),
                )
            wqh_dma(h + 3)
            state[h] = {"qp": qp}

        def stage_b(h):  # rope (straight from psum) -> qT
            qT = work.tile([HD, T], MM_DT, tag="qT", bufs=2, name=f"qT{h}")
            rope(state[h]["qp"][:], qT[:], f"q{h}")
            state[h]["qT"] = qT

        def stage_c1(h):  # scoresT matmuls, exp, mask (per s-tile bank)
            j = h // REP
            st_a, st_b = half_pair(f"sT{h}")
            sts = [st_a[:, 0, :], st_a[:, 1, :], st_b[:, 0, :], st_b[:, 1, :]]
            qT = state[h]["qT"]
            for i in range(TT):
                lo = 128 * i if causal else 0
                mm(
                    sts[i][:, lo:],
                    kT_sb[:, j, _s(i, 128)],
                    qT[:, lo:],
                    start=True,
                    stop=True,
                )
            expm = work.tile([128, TT, T], MM_DT, tag="expm", bufs=2, name=f"expm{h}")
            for i in range(TT):
                lo = 128 * i if causal else 0
                nc.scalar.activation(
                    expm[:, i, lo:], sts[i][:, lo:], AF.Exp, scale=SCALE
                )
                if causal:
                    # only the diagonal 128x128 block is partially masked;
                    # t < lo is never read downstream, t >= lo+128 is fully
                    # visible; the upper-tri pattern is the same for every
                    # diagonal block
                    nc.vector.tensor_mul(
                        expm[:, i, lo : lo + 128],
                        expm[:, i, lo : lo + 128],
                        maskT_sb[:, :],
                    )
                else:
                    nc.vector.tensor_mul(
                        expm[:, i, :], expm[:, i, :], maskT_sb[:, i, :]
                    )
            state[h]["expm"] = expm

        def stage_c2(h):  # AV + normalizer matmuls, reciprocal, scale into aout
            j = h // REP
            expm = state[h]["expm"]
            avp = ps.tile([HD, T], F32, tag="av", bufs=1, name=f"avp{h}")
            for i in range(TT):
                lo = 128 * i if causal else 0
                mm(
                    avp[:, lo:],
                    v_sb[:, i, _s(j, HD)],
                    expm[:, i, lo:],
                    start=(i == 0),
                    stop=(i == TT - 1),
                )
            # normalizer: assemble the column-wise running sum of the s-tiles
            # on the DVE into one contiguous [128, T] tile, then a single
            # N=512 ones-matmul does the partition reduction
            lp = ps.tile([128, T], F32, tag="lsum", bufs=1, name=f"lp{h}")
            esum = work.tile([128, T], MM_DT, tag="esum", bufs=2, name=f"esum{h}")
            if causal:
                tmp = work.tile([128, 256], MM_DT, tag="etmp", name=f"etmp{h}")
                tmp2 = work.tile([128, 128], MM_DT, tag="etmp2", name=f"etmp2{h}")
                nc.vector.tensor_copy(esum[:, 0:128], expm[:, 0, 0:128])
                nc.vector.tensor_add(
                    esum[:, 128:256], expm[:, 0, 128:256], expm[:, 1, 128:256]
                )
                nc.vector.tensor_add(tmp[:], expm[:, 0, 256:], expm[:, 1, 256:])
                nc.vector.tensor_add(
                    esum[:, 256:384], tmp[:, 0:128], expm[:, 2, 256:384]
                )
                nc.vector.tensor_add(tmp2[:], tmp[:, 128:], expm[:, 2, 384:])
                nc.vector.tensor_add(esum[:, 384:], tmp2[:], expm[:, 3, 384:])
            else:
                ea = work.tile([128, T], MM_DT, tag="ea", name=f"ea{h}")
                nc.vector.tensor_add(ea[:], expm[:, 0, :], expm[:, 1, :])
                eb = work.tile([128, T], MM_DT, tag="eb", name=f"eb{h}")
                nc.vector.tensor_add(eb[:], expm[:, 2, :], expm[:, 3, :])
                nc.vector.tensor_add(esum[:], ea[:], eb[:])
            mm(lp[:], ones_sb[:], esum[:], start=True, stop=True)
            recip = work.tile([HD, T], F32, tag="recip", name=f"recip{h}")
            nc.vector.reciprocal_approx_fast(recip[:], lp[:HD, :])
            nc.vector.tensor_mul(aout_sb[:, h, :], avp[:], recip[:])
            del state[h]

        for it in range(NH + 3):
            if it < NH:
                stage_a(it)
            if it < 2:
                rope(kps[it + 2][:], kT_sb[:, it + 2, :], f"k{it + 2}")
            if 0 <= it - 3 < NH:
                stage_c2(it - 3)
            if it < NH:
                stage_b(it)
            if 0 <= it - 2 < NH:
                stage_c1(it - 2)

        # ---- phase C: output projection y = aout.T @ wo ----
        # eg order: the qp/av/lsum-bank group first (those banks free during
        # the phase-B tail, so its matmuls can fill phase-B bubbles; the
        # "sthalf" banks only free after the last exp). The last two egs run
        # i-outer so each t-tile's copy + store DMA overlaps the remaining
        # t-tiles' matmuls instead of draining after the final matmul; y is
        # written in MM_DT (fp16) and cast back on the host.
        def yout(eg, i, yslice, final_eg):
            ysb_i = work.tile(
                [128, 512], MM_DT, tag="ysb1", bufs=8, name=f"ysb{eg}_{i}"
            )
            if i % 2 == 0:
                nc.scalar.copy(ysb_i[:], yslice)
            else:
                nc.vector.tensor_copy(ysb_i[:], yslice)
            if final_eg:
                eng = (nc.scalar, nc.sync, nc.gpsimd, nc.sync)[i]
            else:
                eng = nc.scalar if i % 2 == 0 else nc.sync
            eng.dma_start(y[_s(i, 128), _s(eg, 512)], ysb_i[:])

        for pos, eg in enumerate((1, 0, 3, 2)):
            drain = pos >= 2  # i-outer + interleaved store
            if eg % 2 == 0:
                yp_a, yp_b = half_pair(f"yp{eg}")
                yslices = [yp_a[:, 0, :], yp_a[:, 1, :], yp_b[:, 0, :], yp_b[:, 1, :]]
            else:
                yts = [
                    ps.tile([128, 512], F32, tag=t, bufs=bf, name=f"yp{eg}_{i}")
                    for i, (t, bf) in enumerate(
                        (("qp", 2), ("qp", 2), ("av", 1), ("lsum", 1))
                    )
                ]
                yslices = [t[:] for t in yts]
            wots = []
            for fg in range(4):
                wot = streams.tile(
                    [128, 4, 512], MM_DT, tag="wot", bufs=8, name=f"wot{eg}_{fg}"
                )
                (nc.sync if fg % 2 == 0 else nc.scalar).dma_start(
                    wot[:],
                    wo[_s(fg, 512), _s(eg, 512)].rearrange("(c p) e -> p c e", p=128),
                )
                wots.append(wot)
                if not drain:
                    for ci in range(4):
                        ft = fg * 4 + ci
                        for i in range(TT):
                            mm(
                                yslices[i],
                                aout_sb[:, ft, _s(i, 128)],
                                wot[:, ci, :],
                                start=(ft == 0),
                                stop=(ft == CT - 1),
                            )
            if drain:
                final = pos == 3
                for i in range(TT):
                    if final and i == TT - 1:
                        # column-split the very last chain so the first
                        # half's copy + store overlaps the second half's
                        # matmuls, shrinking the post-matmul drain
                        ysb = work.tile(
                            [128, 512], MM_DT, tag="ysb1", bufs=8, name=f"ysbF{i}"
                        )
                        for half in range(2):
                            cols = slice(256 * half, 256 * (half + 1))
                            for fg in range(4):
                                for ci in range(4):
                                    ft = fg * 4 + ci
                                    mm(
                                        yslices[i][:, cols],
                                        aout_sb[:, ft, _s(i, 128)],
                                        wots[fg][:, ci, cols],
                                        start=(ft == 0),
                                        stop=(ft == CT - 1),
                                    )
                            if half == 0:
                                nc.scalar.copy(ysb[:, cols], yslices[i][:, cols])
                                nc.scalar.dma_start(
                                    y[_s(i, 128), eg * 512 : eg * 512 + 256],
                                    ysb[:, 0:256],
                                )
                            else:
                                nc.vector.tensor_copy(ysb[:, cols], yslices[i][:, cols])
                                nc.sync.dma_start(
                                    y[_s(i, 128), eg * 512 + 256 : (eg + 1) * 512],
                                    ysb[:, 256:512],
                                )
                    else:
                        for fg in range(4):
                            for ci in range(4):
                                ft = fg * 4 + ci
                                mm(
                                    yslices[i],
                                    aout_sb[:, ft, _s(i, 128)],
                                    wots[fg][:, ci, :],
                                    start=(ft == 0),
                                    stop=(ft == CT - 1),
                                )
                        yout(eg, i, yslices[i], final)
            else:
                for i in range(TT):
                    yout(eg, i, yslices[i], False)


def build(causal=False):
    nc = bacc.Bacc(
        "TRN2",
        target_bir_lowering=False,
        debug=False,
        enable_asserts=False,
        num_devices=N_CORES,
    )
    xT = nc.dram_tensor("xT", [C, T], MM_DT, kind="ExternalInput").ap()
    wq = nc.dram_tensor("wq", [C, C], MM_DT, kind="ExternalInput").ap()
    wk = nc.dram_tensor("wk", [C, NKV * HD], MM_DT, kind="ExternalInput").ap()
    wv = nc.dram_tensor("wv", [C, NKV * HD], MM_DT, kind="ExternalInput").ap()
    wo = nc.dram_tensor("wo", [C, C], MM_DT, kind="ExternalInput").ap()
    cosT = nc.dram_tensor("cosT", [HD, T], MM_DT, kind="ExternalInput").ap()
    sinT = nc.dram_tensor("sinT", [HD, T], MM_DT, kind="ExternalInput").ap()
    maskT = nc.dram_tensor(
        "maskT", [128, 128] if causal else [T, T], MM_DT, kind="ExternalInput"
    ).ap()
    y = nc.dram_tensor("y", [T, C], MM_DT, kind="ExternalOutput").ap()

    with tile.TileContext(nc) as tc:
        _emit(tc, xT, wq, wk, wv, wo, cosT, sinT, maskT, y, causal)
    nc.compile()
    return nc


_NC = {}


def _get_nc(causal):
    if causal not in _NC:
        _NC[causal] = build(causal)
    return _NC[causal]


def _is_causal(mask):
    return bool(np.array_equal(mask, np.tril(np.ones((T, T), dtype=bool))))


def host_tables():
    """cos/sin tables (transposed) and the signed rotate-half matrix."""
    inv = 1.0 / (10000.0 ** (np.arange(0, HD, 2, dtype=np.float32) / HD))
    t = np.arange(T, dtype=np.float32)
    freqs = np.outer(t, inv)  # [T, HD/2]
    emb = np.concatenate([freqs, freqs], axis=-1)  # [T, HD]
    cosT = np.ascontiguousarray(np.cos(emb).T, dtype=np.float32)
    sinT = np.ascontiguousarray(np.sin(emb).T, dtype=np.float32)
    # rotate-half signs baked in: rows d<64 multiply the shifted-down half
    # with a minus sign (q'[d] = q[d]cos - q[d+64]sin for d<64)
    sinT[: HD // 2] *= -1.0
    return cosT, sinT


def make_in_maps(inputs, causal=None):
    x = np.asarray(inputs["x"], dtype=np.float32)
    mask = np.asarray(inputs["mask"]).reshape(T, T)
    if causal is None:
        causal = _is_causal(mask)
    cosT, sinT = host_tables()
    if causal:
        # every diagonal 128x128 block of the causal [s,t] mask is the same
        # upper triangle
        maskT = np.triu(np.ones((128, 128), dtype=MM_NP))
    else:
        maskT = np.ascontiguousarray(mask.T).astype(MM_NP)  # [s, t]
    shared = {
        "wq": np.ascontiguousarray(np.asarray(inputs["wq"]).astype(MM_NP)),
        "wk": np.ascontiguousarray(np.asarray(inputs["wk"]).astype(MM_NP)),
        "wv": np.ascontiguousarray(np.asarray(inputs["wv"]).astype(MM_NP)),
        "wo": np.ascontiguousarray(np.asarray(inputs["wo"]).astype(MM_NP)),
        "cosT": cosT.astype(MM_NP),
        "sinT": sinT.astype(MM_NP),
        "maskT": maskT,
    }
    return [
        {"xT": np.ascontiguousarray(x[b].T).astype(MM_NP), **shared}
        for b in range(N_CORES)
    ]


def run(inputs, **kw):
    mask = np.asarray(inputs["mask"]).reshape(T, T)
    causal = _is_causal(mask)
    nc = _get_nc(causal)
    in_maps = make_in_maps(inputs, causal)
    res = run_bass_kernel_spmd(nc, in_maps, core_ids=list(range(N_CORES)), **kw)
    out = np.stack([r["y"] for r in res.results], axis=0).astype(np.float32)
    return out, res


def kernel(**inputs) -> np.ndarray:
    out, _ = run(inputs)
    return out


# revision 6
# speedup vs baseline: 1.0008x; 1.0008x over previous
# Trainium2 Bass kernel for GQA attention block (B=8, T=512, C=2048, 16 q heads,
# 4 kv heads, head_dim=128, RoPE, causal mask, output projection).
#
# Strategy: data parallel across the 8 NeuronCores — core i handles batch
# element i with the full weight set (no collectives). Per core everything is
# computed in a transposed layout:
#   qT/kT [d, t]  = W[:, d-tile].T-contract  (lhsT = weight tile, rhs = xT)
#   scoresT [s,t] = kT-slice.T @ qT          (softmax normalizer along the
#                                             partition dim via a ones-matmul
#                                             that broadcasts the sum to all
#                                             128 partitions for free)
#   outT [d, t]   = v-slice.T @ (mask*exp(scoresT))  (unnormalized)
#   y [t, e]      = outT-slice.T @ wo-tile   (normalized outT)
# RoPE rotate-half is a signed 128x128 permutation matmul + 2 muls + 1 add.
# Matmul operands are fp16 (full PE rate, half the HBM bytes of fp32, DVE 2x
# modes; fp32 PSUM accumulation throughout).
#
# Phase A streams v-proj and k-proj interleaved per 128-row contraction tile:
# each ct consumes x[ct] (scalar ring) + wv[ct] (sync ring) + wk[ct] (gpsimd
# ring) — ~226 GB/s aggregate, under the 358 GB/s per-core HBM cap, with each
# 128KB chunk arriving just-in-time (k lags v by 2 cts to cover the slower
# SWDGE first-chunk latency). kv heads 0/1 finish their contraction two cts
# before heads 2/3, so rope-k0/k1 (DVE) runs under the k23 tail matmuls and
# the first q-head projection starts with no PE bubble. When the runtime mask
# is exactly causal, the scores/AV/normalizer matmuls restrict their moving
# dim to the visible t-range and only the diagonal 128x128 block gets
# mask-multiplied; any other mask falls back to a general masked build.

import os
import sys

import numpy as np

for _p in (
    "/root/.axon_site",
    "/root/.axon_site/_ro/trn_rl_repo",
    "/root/.axon_site/_ro/pypackages",
    "/opt/trn_rl_repo",
):
    if os.path.isdir(_p) and _p not in sys.path:
        sys.path.append(_p)

import concourse.bass as bass  # noqa: E402
import concourse.mybir as mybir  # noqa: E402
import concourse.tile as tile  # noqa: E402
from concourse import bacc  # noqa: E402
from concourse.bass_utils import run_bass_kernel_spmd  # noqa: E402

F32 = mybir.dt.float32
F32R = mybir.dt.float32r
F16 = mybir.dt.float16
AF = mybir.ActivationFunctionType

B, T, C = 8, 512, 2048
HD, NH, NKV = 128, 16, 4
CT = C // 128  # 16 contraction tiles
TT = T // 128  # 4 t/s tiles
EG = C // 512  # 4 output column groups
REP = NH // NKV
SCALE = float(HD) ** -0.5
N_CORES = 8

_DT_ENV = os.environ.get("ATTN_DTYPE", "fp16")
MM_DT = {"fp16": F16, "fp32r": F32R, "fp32": F32}[_DT_ENV]
MM_NP = {"fp16": np.float16, "fp32r": np.float32, "fp32": np.float32}[_DT_ENV]


def _s(i, n):
    return slice(i * n, (i + 1) * n)


def _emit(tc, xT, wq, wk, wv, wo, cosT, sinT, maskT, y, causal):
    nc = tc.nc
    mm = nc.tensor.matmul

    with (
        tc.tile_pool(name="consts", bufs=1) as consts,
        tc.tile_pool(name="streams", bufs=2) as streams,
        tc.tile_pool(name="work", bufs=2) as work,
        tc.tile_pool(name="ps", bufs=1, space="PSUM") as ps,
    ):
        cosT_sb = consts.tile([HD, T], MM_DT)
        sinT_sb = consts.tile([HD, T], MM_DT)
        # causal: one [128,128] upper-tri block reused for every diagonal
        # tile; general: the full [s,t] mask
        maskT_sb = consts.tile([128, 128] if causal else [128, TT, T], MM_DT)
        ones_sb = consts.tile([128, 128], MM_DT)
        xT_sb = consts.tile([128, CT, T], MM_DT)
        kT_sb = consts.tile([HD, NKV, T], MM_DT)
        v_sb = consts.tile([128, TT, 4 * HD], MM_DT)
        aout_sb = consts.tile([HD, NH, T], MM_DT)

        # ---- PE warmup: the HAM clock gate keeps the PE at half rate until
        # a ~3.4us activity window of sustained matmuls. A short burst covers
        # the DMA-latency head (first real operands land ~1.5us after issue);
        # from there the gapless phase-A stream keeps the window busy.
        # Memsets ride the (otherwise idle) DVE so the gpsimd queue can start
        # issuing its DMA stream immediately.
        warm_sb = consts.tile([128, 256], MM_DT)
        nc.vector.memset(warm_sb[:], 0.0)
        nc.vector.memset(ones_sb[:], 1.0)
        wps = ps.tile([HD, T], F32, tag="qp", bufs=2, name="warmps")
        for _ in range(8):
            mm(wps[:, :256], warm_sb[:, :128], warm_sb[:], start=True, stop=True)

        # ---- DMA schedule: one stream per ring, chunks sized so each lands
        # just before its consumption point (a DMA's semaphore only fires
        # when the WHOLE chunk lands; early chunks are single-ct).
        # scalar ring: x, then the first wq heads.
        for c0, ncs in ((0, 1), (1, 1), (2, 1), (3, 1), (4, 1), (5, 1), (6, 2), (8, 4), (12, 4)):
            nc.scalar.dma_start(
                xT_sb[:, c0 : c0 + ncs, :],
                xT[128 * c0 : 128 * (c0 + ncs), :].rearrange("(c p) t -> p c t", p=128),
            )

        # sync ring: wv per-ct chunks.
        wvt = {}
        for c0, ncs in ((0, 1), (1, 1), (2, 1), (3, 1), (4, 1), (5, 1), (6, 2), (8, 2), (10, 2), (12, 2), (14, 2)):
            t = streams.tile([128, 2, 4 * HD], MM_DT, tag="wv", bufs=11, name=f"wv{c0}")
            nc.sync.dma_start(
                t[:, :ncs, :],
                wv[c0 * 128 : (c0 + ncs) * 128, :].rearrange("(c p) d -> p c d", p=128),
            )
            for ci in range(ncs):
                wvt[c0 + ci] = (t, ci)

        # gpsimd ring: wk in two column-half streams — kv heads 0/1 first so
        # their contraction (and rope-k0/k1) finishes well before phase A
        # ends — then kv heads 2/3, then the small consts (needed ~25us in).
        wk01t = {}
        for c0, ncs in ((0, 1), (1, 1), (2, 1), (3, 1), (4, 2), (6, 2), (8, 4), (12, 4)):
            t = streams.tile([128, 4, 2 * HD], MM_DT, tag="wk01", bufs=8, name=f"wk01_{c0}")
            nc.gpsimd.dma_start(
                t[:, :ncs, :],
                wk[c0 * 128 : (c0 + ncs) * 128, : 2 * HD].rearrange(
                    "(c p) d -> p c d", p=128
                ),
            )
            for ci in range(ncs):
                wk01t[c0 + ci] = (t, ci)
        wk23t = {}
        for c0, ncs in ((0, 4), (4, 4), (8, 4), (12, 4)):
            t = streams.tile([128, 4, 2 * HD], MM_DT, tag="wk23", bufs=4, name=f"wk23_{c0}")
            nc.gpsimd.dma_start(
                t[:, :ncs, :],
                wk[c0 * 128 : (c0 + ncs) * 128, 2 * HD :].rearrange(
                    "(c p) d -> p c d", p=128
                ),
            )
            for ci in range(ncs):
                wk23t[c0 + ci] = (t, ci)
        nc.gpsimd.dma_start(cosT_sb[:], cosT)
        nc.gpsimd.dma_start(sinT_sb[:], sinT)
        if causal:
            nc.gpsimd.dma_start(maskT_sb[:], maskT)
        else:
            nc.gpsimd.dma_start(
                maskT_sb[:], maskT.rearrange("(st p) t -> p st t", p=128)
            )

        # wq head-slice prefetch, alternating between the two HWDGE rings
        wqh_tiles = {}

        def wqh_dma(h, eng=None):
            if h >= NH:
                return
            wqh = streams.tile([128, CT, HD], MM_DT, tag="wqh", bufs=6, name=f"wqh{h}")
            (eng or (nc.scalar if h % 2 == 0 else nc.sync)).dma_start(
                wqh[:], wq[:, _s(h, HD)].rearrange("(ct p) d -> p ct d", p=128)
            )
            wqh_tiles[h] = wqh

        # ---- rope helper: psrc (PSUM, [d, t] f32) -> out_slice (SBUF) ----
        # rotate-half via partition-shifted PSUM reads; sinT_sb rows 0:64 are
        # pre-negated on the host, so no rotation matmul is needed.
        def rope(psrc, out_slice, tag):
            qrot = work.tile([HD, T], MM_DT, tag="trot", name=f"qrot_{tag}")
            nc.scalar.copy(qrot[0:64, :], psrc[64:128, :])
            nc.scalar.copy(qrot[64:128, :], psrc[0:64, :])
            tcos = work.tile([HD, T], MM_DT, tag="tcos", name=f"tcos_{tag}")
            nc.vector.tensor_mul(tcos[:], psrc, cosT_sb[:])
            nc.vector.tensor_mul(qrot[:], qrot[:], sinT_sb[:])
            nc.vector.tensor_add(out_slice, tcos[:], qrot[:])

        # PSUM "half" tiles: 2 banks each, double-buffered
        def half_pair(name):
            a = ps.tile([128, 2, 512], F32, tag="sthalf", bufs=2, name=f"{name}a")
            b = ps.tile([128, 2, 512], F32, tag="sthalf", bufs=2, name=f"{name}b")
            return a, b

        # ---- phase A: v and k projections interleaved per ct ----
        vp_a, vp_b = half_pair("vp")
        vps = [vp_a[:, 0, :], vp_a[:, 1, :], vp_b[:, 0, :], vp_b[:, 1, :]]
        _kp_tags = (("qp", 2), ("qp", 2), ("av", 1), ("lsum", 1))
        kps = [
            ps.tile([HD, T], F32, tag=t, bufs=bf, name=f"kp{j}")
            for j, (t, bf) in enumerate(_kp_tags)
        ]

        def vmm(ct):
            t, ci = wvt[ct]
            for i in range(TT):
                mm(
                    vps[i],
                    xT_sb[:, ct, _s(i, 128)],
                    t[:, ci, :],
                    start=(ct == 0),
                    stop=(ct == CT - 1),
                )

        def kmm(ct, js):
            tiles = {0: wk01t, 1: wk01t, 2: wk23t, 3: wk23t}
            for j in js:
                t, ci = tiles[j][ct]
                mm(
                    kps[j][:],
                    t[:, ci, _s(j % 2, HD)],
                    xT_sb[:, ct, :],
                    start=(ct == 0),
                    stop=(ct == CT - 1),
                )

        for ct in range(CT):
            vmm(ct)
            if ct >= 2:
                kmm(ct - 2, (0, 1))
            if ct >= 8:
                kmm(ct - 8, (2, 3))
        kmm(14, (0, 1))
        kmm(15, (0, 1))
        # rope k0/k1 (DVE/ACT) runs under the k23 tail matmuls, freeing the
        # qp psum banks well before the first q-head projection needs them
        rope(kps[0][:], kT_sb[:, 0, :], "k0")
        rope(kps[1][:], kT_sb[:, 1, :], "k1")
        for ct in range(8, CT):
            kmm(ct, (2, 3))
        for i in range(TT):
            nc.vector.tensor_copy(v_sb[:, i, :], vps[i])
        # rope k2/k3 directly after the k23 tail (the DVE has slack here),
        # freeing the av/lsum banks before stage_c2(0)
        rope(kps[2][:], kT_sb[:, 2, :], "k2")
        rope(kps[3][:], kT_sb[:, 3, :], "k3")
        for h in range(3):
            wqh_dma(h)

        # ---- phase B: per q head, software pipelined ----
        state = {}

        def stage_a(h):  # projection matmuls into psum
            qp = ps.tile([HD, T], F32, tag="qp", bufs=2, name=f"qp{h}")
            wqh = wqh_tiles.pop(h)
            for ct in range(CT):
                mm(
                    qp[:],
                    wqh[:, ct, :],
                    xT_sb[:, ct, :],
                    start=(ct == 0),
                    stop=(ct == CT - 1),
                )
            wqh_dma(h + 3)
            state[h] = {"qp": qp}

        def stage_b(h):  # rope (straight from psum) -> qT
            qT = work.tile([HD, T], MM_DT, tag="qT", bufs=2, name=f"qT{h}")
            rope(state[h]["qp"][:], qT[:], f"q{h}")
            state[h]["qT"] = qT

        def stage_c1(h):  # scoresT matmuls, exp, mask (per s-tile bank)
            j = h // REP
            st_a, st_b = half_pair(f"sT{h}")
            sts = [st_a[:, 0, :], st_a[:, 1, :], st_b[:, 0, :], st_b[:, 1, :]]
            qT = state[h]["qT"]
            for i in range(TT):
                lo = 128 * i if causal else 0
                mm(
                    sts[i][:, lo:],
                    kT_sb[:, j, _s(i, 128)],
                    qT[:, lo:],
                    start=True,
                    stop=True,
                )
            expm = work.tile([128, TT, T], MM_DT, tag="expm", bufs=2, name=f"expm{h}")
            for i in range(TT):
                lo = 128 * i if causal else 0
                nc.scalar.activation(
                    expm[:, i, lo:], sts[i][:, lo:], AF.Exp, scale=SCALE
                )
                if causal:
                    # only the diagonal 128x128 block is partially masked;
                    # t < lo is never read downstream, t >= lo+128 is fully
                    # visible; the upper-tri pattern is the same for every
                    # diagonal block
                    nc.vector.tensor_mul(
                        expm[:, i, lo : lo + 128],
                        expm[:, i, lo : lo + 128],
                        maskT_sb[:, :],
                    )
                else:
                    nc.vector.tensor_mul(
                        expm[:, i, :], expm[:, i, :], maskT_sb[:, i, :]
                    )
            state[h]["expm"] = expm

        def stage_c2(h):  # AV + normalizer matmuls, reciprocal, scale into aout
            j = h // REP
            expm = state[h]["expm"]
            avp = ps.tile([HD, T], F32, tag="av", bufs=1, name=f"avp{h}")
            for i in range(TT):
                lo = 128 * i if causal else 0
                mm(
                    avp[:, lo:],
                    v_sb[:, i, _s(j, HD)],
                    expm[:, i, lo:],
                    start=(i == 0),
                    stop=(i == TT - 1),
                )
            # normalizer: assemble the column-wise running sum of the s-tiles
            # on the DVE into one contiguous [128, T] tile, then a single
            # N=512 ones-matmul does the partition reduction
            lp = ps.tile([128, T], F32, tag="lsum", bufs=1, name=f"lp{h}")
            esum = work.tile([128, T], MM_DT, tag="esum", bufs=2, name=f"esum{h}")
            if causal:
                tmp = work.tile([128, 256], MM_DT, tag="etmp", name=f"etmp{h}")
                tmp2 = work.tile([128, 128], MM_DT, tag="etmp2", name=f"etmp2{h}")
                nc.vector.tensor_copy(esum[:, 0:128], expm[:, 0, 0:128])
                nc.vector.tensor_add(
                    esum[:, 128:256], expm[:, 0, 128:256], expm[:, 1, 128:256]
                )
                nc.vector.tensor_add(tmp[:], expm[:, 0, 256:], expm[:, 1, 256:])
                nc.vector.tensor_add(
                    esum[:, 256:384], tmp[:, 0:128], expm[:, 2, 256:384]
                )
                nc.vector.tensor_add(tmp2[:], tmp[:, 128:], expm[:, 2, 384:])
                nc.vector.tensor_add(esum[:, 384:], tmp2[:], expm[:, 3, 384:])
            else:
                ea = work.tile([128, T], MM_DT, tag="ea", name=f"ea{h}")
                nc.vector.tensor_add(ea[:], expm[:, 0, :], expm[:, 1, :])
                eb = work.tile([128, T], MM_DT, tag="eb", name=f"eb{h}")
                nc.vector.tensor_add(eb[:], expm[:, 2, :], expm[:, 3, :])
                nc.vector.tensor_add(esum[:], ea[:], eb[:])
            mm(lp[:], ones_sb[:], esum[:], start=True, stop=True)
            recip = work.tile([HD, T], F32, tag="recip", name=f"recip{h}")
            nc.vector.reciprocal_approx_fast(recip[:], lp[:HD, :])
            nc.vector.tensor_mul(aout_sb[:, h, :], avp[:], recip[:])
            del state[h]

        for it in range(NH + 3):
            if it < NH:
                stage_a(it)
            if 0 <= it - 3 < NH:
                stage_c2(it - 3)
            if it < NH:
                stage_b(it)
            if 0 <= it - 2 < NH:
                stage_c1(it - 2)

        # ---- phase C: output projection y = aout.T @ wo ----
        # eg order: the qp/av/lsum-bank group first (those banks free during
        # the phase-B tail, so its matmuls can fill phase-B bubbles; the
        # "sthalf" banks only free after the last exp). The last two egs run
        # i-outer so each t-tile's copy + store DMA overlaps the remaining
        # t-tiles' matmuls instead of draining after the final matmul; y is
        # written in MM_DT (fp16) and cast back on the host.
        def yout(eg, i, yslice, final_eg):
            ysb_i = work.tile(
                [128, 512], MM_DT, tag="ysb1", bufs=8, name=f"ysb{eg}_{i}"
            )
            if i % 2 == 0:
                nc.scalar.copy(ysb_i[:], yslice)
            else:
                nc.vector.tensor_copy(ysb_i[:], yslice)
            if final_eg:
                eng = (nc.scalar, nc.sync, nc.gpsimd, nc.sync)[i]
            else:
                eng = nc.scalar if i % 2 == 0 else nc.sync
            eng.dma_start(y[_s(i, 128), _s(eg, 512)], ysb_i[:])

        for pos, eg in enumerate((1, 0, 3, 2)):
            drain = pos >= 2  # i-outer + interleaved store
            if eg % 2 == 0:
                yp_a, yp_b = half_pair(f"yp{eg}")
                yslices = [yp_a[:, 0, :], yp_a[:, 1, :], yp_b[:, 0, :], yp_b[:, 1, :]]
            else:
                yts = [
                    ps.tile([128, 512], F32, tag=t, bufs=bf, name=f"yp{eg}_{i}")
                    for i, (t, bf) in enumerate(
                        (("qp", 2), ("qp", 2), ("av", 1), ("lsum", 1))
                    )
                ]
                yslices = [t[:] for t in yts]
            wots = []
            for fg in range(4):
                wot = streams.tile(
                    [128, 4, 512], MM_DT, tag="wot", bufs=8, name=f"wot{eg}_{fg}"
                )
                (nc.sync if fg % 2 == 0 else nc.scalar).dma_start(
                    wot[:],
                    wo[_s(fg, 512), _s(eg, 512)].rearrange("(c p) e -> p c e", p=128),
                )
                wots.append(wot)
                if not drain:
                    for ci in range(4):
                        ft = fg * 4 + ci
                        for i in range(TT):
                            mm(
                                yslices[i],
                                aout_sb[:, ft, _s(i, 128)],
                                wot[:, ci, :],
                                start=(ft == 0),
                                stop=(ft == CT - 1),
                            )
            if drain:
                final = pos == 3
                for i in range(TT):
                    if final and i == TT - 1:
                        # column-split the very last chain so the first
                        # half's copy + store overlaps the second half's
                        # matmuls, shrinking the post-matmul drain. The two
                        # halves use SEPARATE psum tiles (the qp/av banks are
                        # free by now) — sharing one tile makes the second
                        # half's matmuls wait on the first half's copy
                        # (tile-granular write-after-read).
                        ysb = work.tile(
                            [128, 512], MM_DT, tag="ysb1", bufs=8, name=f"ysbF{i}"
                        )
                        yfin = [
                            ps.tile([128, 512], F32, tag="qp", bufs=2, name="yfinA"),
                            ps.tile([128, 512], F32, tag="av", bufs=1, name="yfinB"),
                        ]
                        for half in range(2):
                            cols = slice(256 * half, 256 * (half + 1))
                            for fg in range(4):
                                for ci in range(4):
                                    ft = fg * 4 + ci
                                    mm(
                                        yfin[half][:, cols],
                                        aout_sb[:, ft, _s(i, 128)],
                                        wots[fg][:, ci, cols],
                                        start=(ft == 0),
                                        stop=(ft == CT - 1),
                                    )
                            if half == 0:
                                nc.scalar.copy(ysb[:, cols], yfin[0][:, cols])
                                nc.scalar.dma_start(
                                    y[_s(i, 128), eg * 512 : eg * 512 + 256],
                                    ysb[:, 0:256],
                                )
                            else:
                                nc.vector.tensor_copy(ysb[:, cols], yfin[1][:, cols])
                                nc.sync.dma_start(
                                    y[_s(i, 128), eg * 512 + 256 : (eg + 1) * 512],
                                    ysb[:, 256:512],
                                )
                    else:
                        for fg in range(4):
                            for ci in range(4):
                                ft = fg * 4 + ci
                                mm(
                                    yslices[i],
                                    aout_sb[:, ft, _s(i, 128)],
                                    wots[fg][:, ci, :],
                                    start=(ft == 0),
                                    stop=(ft == CT - 1),
                                )
                        yout(eg, i, yslices[i], final)
            else:
                for i in range(TT):
                    yout(eg, i, yslices[i], False)


def build(causal=False):
    nc = bacc.Bacc(
        "TRN2",
        target_bir_lowering=False,
        debug=False,
        enable_asserts=False,
        num_devices=N_CORES,
    )
    xT = nc.dram_tensor("xT", [C, T], MM_DT, kind="ExternalInput").ap()
    wq = nc.dram_tensor("wq", [C, C], MM_DT, kind="ExternalInput").ap()
    wk = nc.dram_tensor("wk", [C, NKV * HD], MM_DT, kind="ExternalInput").ap()
    wv = nc.dram_tensor("wv", [C, NKV * HD], MM_DT, kind="ExternalInput").ap()
    wo = nc.dram_tensor("wo", [C, C], MM_DT, kind="ExternalInput").ap()
    cosT = nc.dram_tensor("cosT", [HD, T], MM_DT, kind="ExternalInput").ap()
    sinT = nc.dram_tensor("sinT", [HD, T], MM_DT, kind="ExternalInput").ap()
    maskT = nc.dram_tensor(
        "maskT", [128, 128] if causal else [T, T], MM_DT, kind="ExternalInput"
    ).ap()
    y = nc.dram_tensor("y", [T, C], MM_DT, kind="ExternalOutput").ap()

    with tile.TileContext(nc) as tc:
        _emit(tc, xT, wq, wk, wv, wo, cosT, sinT, maskT, y, causal)
    nc.compile()
    return nc


_NC = {}


def _get_nc(causal):
    if causal not in _NC:
        _NC[causal] = build(causal)
    return _NC[causal]


def _is_causal(mask):
    return bool(np.array_equal(mask, np.tril(np.ones((T, T), dtype=bool))))


def host_tables():
    """cos/sin tables (transposed) and the signed rotate-half matrix."""
    inv = 1.0 / (10000.0 ** (np.arange(0, HD, 2, dtype=np.float32) / HD))
    t = np.arange(T, dtype=np.float32)
    freqs = np.outer(t, inv)  # [T, HD/2]
    emb = np.concatenate([freqs, freqs], axis=-1)  # [T, HD]
    cosT = np.ascontiguousarray(np.cos(emb).T, dtype=np.float32)
    sinT = np.ascontiguousarray(np.sin(emb).T, dtype=np.float32)
    # rotate-half signs baked in: rows d<64 multiply the shifted-down half
    # with a minus sign (q'[d] = q[d]cos - q[d+64]sin for d<64)
    sinT[: HD // 2] *= -1.0
    return cosT, sinT


def make_in_maps(inputs, causal=None):
    x = np.asarray(inputs["x"], dtype=np.float32)
    mask = np.asarray(inputs["mask"]).reshape(T, T)
    if causal is None:
        causal = _is_causal(mask)
    cosT, sinT = host_tables()
    if causal:
        # every diagonal 128x128 block of the causal [s,t] mask is the same
        # upper triangle
        maskT = np.triu(np.ones((128, 128), dtype=MM_NP))
    else:
        maskT = np.ascontiguousarray(mask.T).astype(MM_NP)  # [s, t]
    shared = {
        "wq": np.ascontiguousarray(np.asarray(inputs["wq"]).astype(MM_NP)),
        "wk": np.ascontiguousarray(np.asarray(inputs["wk"]).astype(MM_NP)),
        "wv": np.ascontiguousarray(np.asarray(inputs["wv"]).astype(MM_NP)),
        "wo": np.ascontiguousarray(np.asarray(inputs["wo"]).astype(MM_NP)),
        "cosT": cosT.astype(MM_NP),
        "sinT": sinT.astype(MM_NP),
        "maskT": maskT,
    }
    return [
        {"xT": np.ascontiguousarray(x[b].T).astype(MM_NP), **shared}
        for b in range(N_CORES)
    ]


def run(inputs, **kw):
    mask = np.asarray(inputs["mask"]).reshape(T, T)
    causal = _is_causal(mask)
    nc = _get_nc(causal)
    in_maps = make_in_maps(inputs, causal)
    res = run_bass_kernel_spmd(nc, in_maps, core_ids=list(range(N_CORES)), **kw)
    out = np.stack([r["y"] for r in res.results], axis=0).astype(np.float32)
    return out, res


def kernel(**inputs) -> np.ndarray:
    out, _ = run(inputs)
    return out


# revision 12
# speedup vs baseline: 1.0285x; 1.0277x over previous
# Trainium2 Bass kernel for GQA attention block (B=8, T=512, C=2048, 16 q heads,
# 4 kv heads, head_dim=128, RoPE, causal mask, output projection).
#
# Strategy: data parallel across the 8 NeuronCores — core i handles batch
# element i with the full weight set (no collectives). Per core everything is
# computed in a transposed layout:
#   qT/kT [d, t]  = W[:, d-tile].T-contract  (lhsT = weight tile, rhs = xT)
#   scoresT [s,t] = kT-slice.T @ qT          (softmax normalizer along the
#                                             partition dim via a ones-matmul
#                                             that broadcasts the sum to all
#                                             128 partitions for free)
#   outT [d, t]   = v-slice.T @ (mask*exp(scoresT))  (unnormalized)
#   y [t, e]      = outT-slice.T @ wo-tile   (normalized outT)
# RoPE rotate-half is a signed 128x128 permutation matmul + 2 muls + 1 add.
# Matmul operands are fp16 (full PE rate, half the HBM bytes of fp32, DVE 2x
# modes; fp32 PSUM accumulation throughout).
#
# Phase A streams v-proj and k-proj interleaved per 128-row contraction tile:
# each ct consumes x[ct] (scalar ring) + wv[ct] (sync ring) + wk[ct] (gpsimd
# ring) — ~226 GB/s aggregate, under the 358 GB/s per-core HBM cap, with each
# 128KB chunk arriving just-in-time (k lags v by 2 cts to cover the slower
# SWDGE first-chunk latency). kv heads 0/1 finish their contraction two cts
# before heads 2/3, so rope-k0/k1 (DVE) runs under the k23 tail matmuls and
# the first q-head projection starts with no PE bubble. When the runtime mask
# is exactly causal, the scores/AV/normalizer matmuls restrict their moving
# dim to the visible t-range and only the diagonal 128x128 block gets
# mask-multiplied; any other mask falls back to a general masked build.

import os
import sys

import numpy as np

for _p in (
    "/root/.axon_site",
    "/root/.axon_site/_ro/trn_rl_repo",
    "/root/.axon_site/_ro/pypackages",
    "/opt/trn_rl_repo",
):
    if os.path.isdir(_p) and _p not in sys.path:
        sys.path.append(_p)

import concourse.bass as bass  # noqa: E402
import concourse.mybir as mybir  # noqa: E402
import concourse.tile as tile  # noqa: E402
from concourse import bacc  # noqa: E402
from concourse.bass_utils import run_bass_kernel_spmd  # noqa: E402

F32 = mybir.dt.float32
F32R = mybir.dt.float32r
F16 = mybir.dt.float16
AF = mybir.ActivationFunctionType

B, T, C = 8, 512, 2048
HD, NH, NKV = 128, 16, 4
CT = C // 128  # 16 contraction tiles
TT = T // 128  # 4 t/s tiles
EG = C // 512  # 4 output column groups
REP = NH // NKV
SCALE = float(HD) ** -0.5
N_CORES = 8

_DT_ENV = os.environ.get("ATTN_DTYPE", "fp16")
MM_DT = {"fp16": F16, "fp32r": F32R, "fp32": F32}[_DT_ENV]
MM_NP = {"fp16": np.float16, "fp32r": np.float32, "fp32": np.float32}[_DT_ENV]


def _s(i, n):
    return slice(i * n, (i + 1) * n)


def _emit(tc, xT, wq, wk01, wk23, wv, wo, cosT, sinT, maskT, y, causal):
    nc = tc.nc
    mm = nc.tensor.matmul

    with (
        tc.tile_pool(name="consts", bufs=1) as consts,
        tc.tile_pool(name="streams", bufs=2) as streams,
        tc.tile_pool(name="work", bufs=2) as work,
        tc.tile_pool(name="ps", bufs=1, space="PSUM") as ps,
    ):
        cosT_sb = consts.tile([HD, T], MM_DT)
        sinT_sb = consts.tile([HD, T], MM_DT)
        # causal: one [128,128] upper-tri block reused for every diagonal
        # tile; general: the full [s,t] mask
        maskT_sb = consts.tile([128, 128] if causal else [128, TT, T], MM_DT)
        ones_sb = consts.tile([128, 128], MM_DT)
        xT_sb = consts.tile([128, CT, T], MM_DT)
        kT_sb = consts.tile([HD, NKV, T], MM_DT)
        v_sb = consts.tile([128, TT, 4 * HD], MM_DT)
        aout_sb = consts.tile([HD, NH, T], MM_DT)

        # ---- PE warmup: the HAM clock gate keeps the PE at half rate until
        # a ~3.4us activity window of sustained matmuls. A short burst covers
        # the DMA-latency head (first real operands land ~1.5us after issue);
        # from there the gapless phase-A stream keeps the window busy.
        # Memsets ride the (otherwise idle) DVE so the gpsimd queue can start
        # issuing its DMA stream immediately.
        warm_sb = consts.tile([128, 256], MM_DT)
        nc.vector.memset(warm_sb[:], 0.0)
        nc.vector.memset(ones_sb[:], 1.0)
        wps = ps.tile([HD, T], F32, tag="qp", bufs=2, name="warmps")
        for _ in range(8):
            mm(wps[:, :256], warm_sb[:, :128], warm_sb[:], start=True, stop=True)

        # ---- DMA schedule. Two hard constraints drive the emission order:
        # (1) DMA completion semaphores come from 8 lanes assigned round-robin
        #     over GLOBAL emission order — DMA #n's issue instruction blocks
        #     its engine until DMA #(n-8) has fully landed. So the first 8
        #     emitted DMAs are the urgent heads of each stream, and the
        #     rotation below keeps every lane collision pointing at a
        #     transfer that completed long before.
        # (2) wq/wk/wv are pre-tiled on the host into SBUF layout so every
        #     transfer is one fat contiguous descriptor per partition.
        # x rides the scalar HWDGE ring, wv the sync ring, wk halves the
        # gpsimd (SWDGE) ring; consts fill gpsimd's fresh-lane slots.
        wvt = {}
        wk01t = {}
        wk23t = {}

        def x_dma(c0, ncs):
            nc.scalar.dma_start(
                xT_sb[:, c0 : c0 + ncs, :],
                xT[128 * c0 : 128 * (c0 + ncs), :].rearrange("(c p) t -> p c t", p=128),
            )

        def wv_dma(c0, ncs):
            t = streams.tile([128, 3, 4 * HD], MM_DT, tag="wv", bufs=8, name=f"wv{c0}")
            nc.sync.dma_start(t[:, :ncs, :], wv[:, c0 * 512 : (c0 + ncs) * 512])
            for ci in range(ncs):
                wvt[c0 + ci] = (t, ci)

        def wk01_dma(c0, ncs):
            t = streams.tile([128, 2, 2 * HD], MM_DT, tag="wk01", bufs=8, name=f"wk01_{c0}")
            nc.gpsimd.dma_start(t[:, :ncs, :], wk01[:, c0 * 256 : (c0 + ncs) * 256])
            for ci in range(ncs):
                wk01t[c0 + ci] = (t, ci)

        def wk23_dma(c0, ncs):
            t = streams.tile([128, 4, 2 * HD], MM_DT, tag="wk23", bufs=4, name=f"wk23_{c0}")
            nc.gpsimd.dma_start(t[:, :ncs, :], wk23[:, c0 * 256 : (c0 + ncs) * 256])
            for ci in range(ncs):
                wk23t[c0 + ci] = (t, ci)

        def mask_dma():
            if causal:
                nc.gpsimd.dma_start(maskT_sb[:], maskT)
            else:
                nc.gpsimd.dma_start(
                    maskT_sb[:], maskT.rearrange("(st p) t -> p st t", p=128)
                )

        # row 0 (fresh lanes): urgent heads of every stream
        x_dma(0, 1)
        wv_dma(0, 1)
        wk01_dma(0, 2)
        nc.gpsimd.dma_start(cosT_sb[:], cosT)
        nc.gpsimd.dma_start(sinT_sb[:], sinT)
        mask_dma()
        x_dma(1, 2)
        wv_dma(1, 2)
        # row 1
        wk01_dma(2, 2)
        x_dma(3, 2)
        wv_dma(3, 2)
        wk23_dma(0, 4)
        x_dma(5, 2)
        wv_dma(5, 2)
        wk01_dma(4, 2)
        wk23_dma(4, 4)
        # row 2
        x_dma(7, 2)
        wv_dma(7, 2)
        wk01_dma(6, 2)
        x_dma(9, 2)
        wv_dma(9, 2)
        wk01_dma(8, 2)
        x_dma(11, 2)
        wv_dma(11, 2)
        # row 3
        wk01_dma(10, 2)
        x_dma(13, 3)
        wv_dma(13, 3)
        wk23_dma(8, 4)
        wk01_dma(12, 2)

        # wq head-slice prefetch (pre-tiled: one 4KB descriptor/partition),
        # alternating between the two HWDGE rings
        wqh_tiles = {}

        def wqh_dma(h, eng=None):
            if h >= NH:
                return
            wqh = streams.tile([128, CT, HD], MM_DT, tag="wqh", bufs=6, name=f"wqh{h}")
            (eng or (nc.scalar if h % 2 == 0 else nc.sync)).dma_start(
                wqh[:], wq[_s(h, 128), :]
            )
            wqh_tiles[h] = wqh

        wqh_dma(0)
        wqh_dma(1)
        wk01_dma(14, 2)
        # row 4
        wk23_dma(12, 4)
        wqh_dma(2)

        # ---- rope helper: psrc (PSUM, [d, t] f32) -> out_slice (SBUF) ----
        # rotate-half via partition-shifted PSUM reads; sinT_sb rows 0:64 are
        # pre-negated on the host, so no rotation matmul is needed.
        def rope(psrc, out_slice, tag):
            qrot = work.tile([HD, T], MM_DT, tag="trot", name=f"qrot_{tag}")
            nc.scalar.copy(qrot[0:64, :], psrc[64:128, :])
            nc.scalar.copy(qrot[64:128, :], psrc[0:64, :])
            tcos = work.tile([HD, T], MM_DT, tag="tcos", name=f"tcos_{tag}")
            nc.vector.tensor_mul(tcos[:], psrc, cosT_sb[:])
            nc.vector.tensor_mul(qrot[:], qrot[:], sinT_sb[:])
            nc.vector.tensor_add(out_slice, tcos[:], qrot[:])

        # PSUM "half" tiles: 2 banks each, double-buffered
        def half_pair(name):
            a = ps.tile([128, 2, 512], F32, tag="sthalf", bufs=2, name=f"{name}a")
            b = ps.tile([128, 2, 512], F32, tag="sthalf", bufs=2, name=f"{name}b")
            return a, b

        # ---- phase A: v and k projections interleaved per ct ----
        vp_a, vp_b = half_pair("vp")
        vps = [vp_a[:, 0, :], vp_a[:, 1, :], vp_b[:, 0, :], vp_b[:, 1, :]]
        _kp_tags = (("qp", 2), ("qp", 2), ("av", 1), ("lsum", 1))
        kps = [
            ps.tile([HD, T], F32, tag=t, bufs=bf, name=f"kp{j}")
            for j, (t, bf) in enumerate(_kp_tags)
        ]

        def vmm(ct):
            t, ci = wvt[ct]
            for i in range(TT):
                mm(
                    vps[i],
                    xT_sb[:, ct, _s(i, 128)],
                    t[:, ci, :],
                    start=(ct == 0),
                    stop=(ct == CT - 1),
                )

        def kmm(ct, js):
            tiles = {0: wk01t, 1: wk01t, 2: wk23t, 3: wk23t}
            for j in js:
                t, ci = tiles[j][ct]
                mm(
                    kps[j][:],
                    t[:, ci, _s(j % 2, HD)],
                    xT_sb[:, ct, :],
                    start=(ct == 0),
                    stop=(ct == CT - 1),
                )

        # k01 lags v by 2 cts early on, then catches up 2-at-a-time so its
        # contraction finishes by the v(12) group — rope-k0/k1 then runs
        # under the v tail + k23 tail with several us of PE cover.
        k01_sched = {8: (6, 7), 9: (8, 9), 10: (10, 11), 11: (12, 13), 12: (14, 15)}
        for ct in range(CT):
            vmm(ct)
            if 2 <= ct < 8:
                kmm(ct - 2, (0, 1))
            for c in k01_sched.get(ct, ()):
                kmm(c, (0, 1))
            if ct >= 6:
                kmm(ct - 6, (2, 3))
        # rope k0/k1 (DVE/ACT) frees the qp psum banks well before the
        # first q-head projection needs them
        rope(kps[0][:], kT_sb[:, 0, :], "k0")
        rope(kps[1][:], kT_sb[:, 1, :], "k1")
        for ct in range(10, CT):
            kmm(ct, (2, 3))
        for i in range(TT):
            nc.vector.tensor_copy(v_sb[:, i, :], vps[i])
        # rope k2/k3 directly after the k23 tail (the DVE has slack here),
        # freeing the av/lsum banks before stage_c2(0)
        rope(kps[2][:], kT_sb[:, 2, :], "k2")
        rope(kps[3][:], kT_sb[:, 3, :], "k3")

        # ---- phase B: per q head, software pipelined ----
        state = {}

        def stage_a(h):  # projection matmuls into psum
            qp = ps.tile([HD, T], F32, tag="qp", bufs=2, name=f"qp{h}")
            wqh = wqh_tiles.pop(h)
            for ct in range(CT):
                mm(
                    qp[:],
                    wqh[:, ct, :],
                    xT_sb[:, ct, :],
                    start=(ct == 0),
                    stop=(ct == CT - 1),
                )
            wqh_dma(h + 3)
            state[h] = {"qp": qp}

        def stage_b(h):  # rope (straight from psum) -> qT
            qT = work.tile([HD, T], MM_DT, tag="qT", bufs=2, name=f"qT{h}")
            rope(state[h]["qp"][:], qT[:], f"q{h}")
            state[h]["qT"] = qT

        def stage_c1(h):  # scoresT matmuls, exp, mask (per s-tile bank)
            j = h // REP
            st_a, st_b = half_pair(f"sT{h}")
            sts = [st_a[:, 0, :], st_a[:, 1, :], st_b[:, 0, :], st_b[:, 1, :]]
            qT = state[h]["qT"]
            for i in range(TT):
                lo = 128 * i if causal else 0
                mm(
                    sts[i][:, lo:],
                    kT_sb[:, j, _s(i, 128)],
                    qT[:, lo:],
                    start=True,
                    stop=True,
                )
            expm = work.tile([128, TT, T], MM_DT, tag="expm", bufs=2, name=f"expm{h}")
            for i in range(TT):
                lo = 128 * i if causal else 0
                nc.scalar.activation(
                    expm[:, i, lo:], sts[i][:, lo:], AF.Exp, scale=SCALE
                )
                if causal:
                    # only the diagonal 128x128 block is partially masked;
                    # t < lo is never read downstream, t >= lo+128 is fully
                    # visible; the upper-tri pattern is the same for every
                    # diagonal block
                    nc.vector.tensor_mul(
                        expm[:, i, lo : lo + 128],
                        expm[:, i, lo : lo + 128],
                        maskT_sb[:, :],
                    )
                else:
                    nc.vector.tensor_mul(
                        expm[:, i, :], expm[:, i, :], maskT_sb[:, i, :]
                    )
            state[h]["expm"] = expm

        def stage_c2(h):  # AV + normalizer matmuls, reciprocal, scale into aout
            j = h // REP
            expm = state[h]["expm"]
            avp = ps.tile([HD, T], F32, tag="av", bufs=1, name=f"avp{h}")
            for i in range(TT):
                lo = 128 * i if causal else 0
                mm(
                    avp[:, lo:],
                    v_sb[:, i, _s(j, HD)],
                    expm[:, i, lo:],
                    start=(i == 0),
                    stop=(i == TT - 1),
                )
            # normalizer: assemble the column-wise running sum of the s-tiles
            # on the DVE into one contiguous [128, T] tile, then a single
            # N=512 ones-matmul does the partition reduction
            lp = ps.tile([128, T], F32, tag="lsum", bufs=1, name=f"lp{h}")
            esum = work.tile([128, T], MM_DT, tag="esum", bufs=2, name=f"esum{h}")
            if causal:
                tmp = work.tile([128, 256], MM_DT, tag="etmp", name=f"etmp{h}")
                tmp2 = work.tile([128, 128], MM_DT, tag="etmp2", name=f"etmp2{h}")
                nc.vector.tensor_copy(esum[:, 0:128], expm[:, 0, 0:128])
                nc.vector.tensor_add(
                    esum[:, 128:256], expm[:, 0, 128:256], expm[:, 1, 128:256]
                )
                nc.vector.tensor_add(tmp[:], expm[:, 0, 256:], expm[:, 1, 256:])
                nc.vector.tensor_add(
                    esum[:, 256:384], tmp[:, 0:128], expm[:, 2, 256:384]
                )
                nc.vector.tensor_add(tmp2[:], tmp[:, 128:], expm[:, 2, 384:])
                nc.vector.tensor_add(esum[:, 384:], tmp2[:], expm[:, 3, 384:])
            else:
                ea = work.tile([128, T], MM_DT, tag="ea", name=f"ea{h}")
                nc.vector.tensor_add(ea[:], expm[:, 0, :], expm[:, 1, :])
                eb = work.tile([128, T], MM_DT, tag="eb", name=f"eb{h}")
                nc.vector.tensor_add(eb[:], expm[:, 2, :], expm[:, 3, :])
                nc.vector.tensor_add(esum[:], ea[:], eb[:])
            mm(lp[:], ones_sb[:], esum[:], start=True, stop=True)
            recip = work.tile([HD, T], F32, tag="recip", name=f"recip{h}")
            nc.vector.reciprocal_approx_fast(recip[:], lp[:HD, :])
            nc.vector.tensor_mul(aout_sb[:, h, :], avp[:], recip[:])
            del state[h]

        for it in range(NH + 3):
            if it < NH:
                stage_a(it)
            if 0 <= it - 3 < NH:
                stage_c2(it - 3)
            if it < NH:
                stage_b(it)
            if 0 <= it - 2 < NH:
                stage_c1(it - 2)

        # ---- phase C: output projection y = aout.T @ wo ----
        # eg order: the qp/av/lsum-bank group first (those banks free during
        # the phase-B tail, so its matmuls can fill phase-B bubbles; the
        # "sthalf" banks only free after the last exp). The last two egs run
        # i-outer so each t-tile's copy + store DMA overlaps the remaining
        # t-tiles' matmuls instead of draining after the final matmul; y is
        # written in MM_DT (fp16) and cast back on the host.
        def yout(eg, i, yslice, final_eg):
            ysb_i = work.tile(
                [128, 512], MM_DT, tag="ysb1", bufs=8, name=f"ysb{eg}_{i}"
            )
            if i % 2 == 0:
                nc.scalar.copy(ysb_i[:], yslice)
            else:
                nc.vector.tensor_copy(ysb_i[:], yslice)
            if final_eg:
                eng = (nc.scalar, nc.sync, nc.gpsimd, nc.sync)[i]
            else:
                eng = nc.scalar if i % 2 == 0 else nc.sync
            eng.dma_start(y[_s(i, 128), _s(eg, 512)], ysb_i[:])

        for pos, eg in enumerate((1, 0, 3, 2)):
            drain = pos >= 2  # i-outer + interleaved store
            if eg % 2 == 0:
                yp_a, yp_b = half_pair(f"yp{eg}")
                yslices = [yp_a[:, 0, :], yp_a[:, 1, :], yp_b[:, 0, :], yp_b[:, 1, :]]
            else:
                yts = [
                    ps.tile([128, 512], F32, tag=t, bufs=bf, name=f"yp{eg}_{i}")
                    for i, (t, bf) in enumerate(
                        (("qp", 2), ("qp", 2), ("av", 1), ("lsum", 1))
                    )
                ]
                yslices = [t[:] for t in yts]
            wots = []
            for fg in range(4):
                wot = streams.tile(
                    [128, 4, 512], MM_DT, tag="wot", bufs=8, name=f"wot{eg}_{fg}"
                )
                (nc.sync if fg % 2 == 0 else nc.scalar).dma_start(
                    wot[:],
                    wo[_s(fg, 512), _s(eg, 512)].rearrange("(c p) e -> p c e", p=128),
                )
                wots.append(wot)
                if not drain:
                    for ci in range(4):
                        ft = fg * 4 + ci
                        for i in range(TT):
                            mm(
                                yslices[i],
                                aout_sb[:, ft, _s(i, 128)],
                                wot[:, ci, :],
                                start=(ft == 0),
                                stop=(ft == CT - 1),
                            )
            if drain:
                final = pos == 3
                for i in range(TT):
                    if final and i == TT - 1:
                        # column-split the very last chain so the first
                        # half's copy + store overlaps the second half's
                        # matmuls, shrinking the post-matmul drain. The two
                        # halves use SEPARATE psum tiles (the qp/av banks are
                        # free by now) — sharing one tile makes the second
                        # half's matmuls wait on the first half's copy
                        # (tile-granular write-after-read).
                        ysb = work.tile(
                            [128, 512], MM_DT, tag="ysb1", bufs=8, name=f"ysbF{i}"
                        )
                        yfin = [
                            ps.tile([128, 512], F32, tag="qp", bufs=2, name="yfinA"),
                            ps.tile([128, 512], F32, tag="av", bufs=1, name="yfinB"),
                        ]
                        for half in range(2):
                            cols = slice(256 * half, 256 * (half + 1))
                            for fg in range(4):
                                for ci in range(4):
                                    ft = fg * 4 + ci
                                    mm(
                                        yfin[half][:, cols],
                                        aout_sb[:, ft, _s(i, 128)],
                                        wots[fg][:, ci, cols],
                                        start=(ft == 0),
                                        stop=(ft == CT - 1),
                                    )
                            if half == 0:
                                nc.scalar.copy(ysb[:, cols], yfin[0][:, cols])
                                nc.scalar.dma_start(
                                    y[_s(i, 128), eg * 512 : eg * 512 + 256],
                                    ysb[:, 0:256],
                                )
                            else:
                                nc.vector.tensor_copy(ysb[:, cols], yfin[1][:, cols])
                                nc.sync.dma_start(
                                    y[_s(i, 128), eg * 512 + 256 : (eg + 1) * 512],
                                    ysb[:, 256:512],
                                )
                    else:
                        for fg in range(4):
                            for ci in range(4):
                                ft = fg * 4 + ci
                                mm(
                                    yslices[i],
                                    aout_sb[:, ft, _s(i, 128)],
                                    wots[fg][:, ci, :],
                                    start=(ft == 0),
                                    stop=(ft == CT - 1),
                                )
                        yout(eg, i, yslices[i], final)
            else:
                for i in range(TT):
                    yout(eg, i, yslices[i], False)


def build(causal=False):
    nc = bacc.Bacc(
        "TRN2",
        target_bir_lowering=False,
        debug=False,
        enable_asserts=False,
        num_devices=N_CORES,
    )
    xT = nc.dram_tensor("xT", [C, T], MM_DT, kind="ExternalInput").ap()
    # wq/wk/wv pre-tiled on the host into SBUF layout (one contiguous
    # descriptor per partition per transfer); wk split into kv-head halves
    wq = nc.dram_tensor("wq", [NH * 128, CT * HD], MM_DT, kind="ExternalInput").ap()
    wk01 = nc.dram_tensor("wk01", [128, CT * 2 * HD], MM_DT, kind="ExternalInput").ap()
    wk23 = nc.dram_tensor("wk23", [128, CT * 2 * HD], MM_DT, kind="ExternalInput").ap()
    wv = nc.dram_tensor("wv", [128, CT * 4 * HD], MM_DT, kind="ExternalInput").ap()
    wo = nc.dram_tensor("wo", [C, C], MM_DT, kind="ExternalInput").ap()
    cosT = nc.dram_tensor("cosT", [HD, T], MM_DT, kind="ExternalInput").ap()
    sinT = nc.dram_tensor("sinT", [HD, T], MM_DT, kind="ExternalInput").ap()
    maskT = nc.dram_tensor(
        "maskT", [128, 128] if causal else [T, T], MM_DT, kind="ExternalInput"
    ).ap()
    y = nc.dram_tensor("y", [T, C], MM_DT, kind="ExternalOutput").ap()

    with tile.TileContext(nc) as tc:
        _emit(tc, xT, wq, wk01, wk23, wv, wo, cosT, sinT, maskT, y, causal)
    nc.compile()
    return nc


_NC = {}


def _get_nc(causal):
    if causal not in _NC:
        _NC[causal] = build(causal)
    return _NC[causal]


def _is_causal(mask):
    return bool(np.array_equal(mask, np.tril(np.ones((T, T), dtype=bool))))


def host_tables():
    """cos/sin tables (transposed) and the signed rotate-half matrix."""
    inv = 1.0 / (10000.0 ** (np.arange(0, HD, 2, dtype=np.float32) / HD))
    t = np.arange(T, dtype=np.float32)
    freqs = np.outer(t, inv)  # [T, HD/2]
    emb = np.concatenate([freqs, freqs], axis=-1)  # [T, HD]
    cosT = np.ascontiguousarray(np.cos(emb).T, dtype=np.float32)
    sinT = np.ascontiguousarray(np.sin(emb).T, dtype=np.float32)
    # rotate-half signs baked in: rows d<64 multiply the shifted-down half
    # with a minus sign (q'[d] = q[d]cos - q[d+64]sin for d<64)
    sinT[: HD // 2] *= -1.0
    return cosT, sinT


def make_in_maps(inputs, causal=None):
    x = np.asarray(inputs["x"], dtype=np.float32)
    mask = np.asarray(inputs["mask"]).reshape(T, T)
    if causal is None:
        causal = _is_causal(mask)
    cosT, sinT = host_tables()
    if causal:
        # every diagonal 128x128 block of the causal [s,t] mask is the same
        # upper triangle
        maskT = np.triu(np.ones((128, 128), dtype=MM_NP))
    else:
        maskT = np.ascontiguousarray(mask.T).astype(MM_NP)  # [s, t]
    # pre-tile wq/wk/wv into SBUF layout: dram[p, ct, d] = w[ct*128+p, d]
    # (wq additionally head-major: dram[h*128+p, ct*HD+d] = wq[ct*128+p, h*HD+d])
    wq_f = np.asarray(inputs["wq"], dtype=np.float32)
    wk_f = np.asarray(inputs["wk"], dtype=np.float32)
    wv_f = np.asarray(inputs["wv"], dtype=np.float32)
    wq_t = (
        wq_f.reshape(CT, 128, NH, HD)
        .transpose(2, 1, 0, 3)
        .reshape(NH * 128, CT * HD)
    )
    wk01_t = wk_f[:, : 2 * HD].reshape(CT, 128, 2 * HD).transpose(1, 0, 2).reshape(128, -1)
    wk23_t = wk_f[:, 2 * HD :].reshape(CT, 128, 2 * HD).transpose(1, 0, 2).reshape(128, -1)
    wv_t = wv_f.reshape(CT, 128, 4 * HD).transpose(1, 0, 2).reshape(128, -1)
    shared = {
        "wq": np.ascontiguousarray(wq_t.astype(MM_NP)),
        "wk01": np.ascontiguousarray(wk01_t.astype(MM_NP)),
        "wk23": np.ascontiguousarray(wk23_t.astype(MM_NP)),
        "wv": np.ascontiguousarray(wv_t.astype(MM_NP)),
        "wo": np.ascontiguousarray(np.asarray(inputs["wo"]).astype(MM_NP)),
        "cosT": cosT.astype(MM_NP),
        "sinT": sinT.astype(MM_NP),
        "maskT": maskT,
    }
    return [
        {"xT": np.ascontiguousarray(x[b].T).astype(MM_NP), **shared}
        for b in range(N_CORES)
    ]


def run(inputs, **kw):
    mask = np.asarray(inputs["mask"]).reshape(T, T)
    causal = _is_causal(mask)
    nc = _get_nc(causal)
    in_maps = make_in_maps(inputs, causal)
    res = run_bass_kernel_spmd(nc, in_maps, core_ids=list(range(N_CORES)), **kw)
    out = np.stack([r["y"] for r in res.results], axis=0).astype(np.float32)
    return out, res


def kernel(**inputs) -> np.ndarray:
    out, _ = run(inputs)
    return out
